# revision 21
# baseline (speedup 1.0000x reference)
"""MoE FFN (top-2 of 8 experts) Trainium2 kernel.

Strategy (expert-parallel across 8 NeuronCores):
  - Host computes the (tiny) router: logits = x@Wg, softmax, top-2,
    renormalized combine weights.  Tokens are gathered per expert on the
    host ("all-to-all dispatch" done at sharding time), transposed to
    [H, C] so both FFN GEMMs run with natural weight layouts on device.
  - Core e runs the FFN for expert e over its C_pad gathered tokens,
    F-quarter by F-quarter (quarter weights stream through SBUF,
    double-buffered; chunks of <=512 tokens bound PSUM/SBUF usage):
        hT = gelu_tanh(W1.T-tiles @ xT)        # [Fq, C] per quarter
        Y_fb = hT-tiles.T @ W2_fb              # [C, H] partial per quarter
    Partials land in per-quarter DRAM regions; the host sums them
    (cheaper than device-side DRAM read-back accumulation).
  - Host applies combine weights + b2 and scatter-adds back ("combine").

  All matmuls use float32r (full-rate fp32 tensor-engine mode, fp32
  storage, fp32 PSUM accumulation).  x/W1/W2 are staged in DRAM as
  bf16 and cast to fp32 by the DMA on the way into SBUF (gpsimd SWDGE
  cast-DMA) — this halves HBM read traffic, which otherwise starves
  the PE during the first weight-block.  Partial outputs return as
  bf16 (summed in fp32 on the host), halving the tail flush.

The kernel is compiled once per (C_pad, chunk-structure, biases-zero)
configuration and cached in-process.
"""

import os
import sys
import numpy as np

for _p in ("/opt/trn_rl_repo", "/root/.axon_site/_ro/trn_rl_repo"):
    if _p not in sys.path and os.path.isdir(_p):
        sys.path.append(_p)

import concourse.bacc as bacc  # noqa: E402
import concourse.tile as tile  # noqa: E402
from concourse import mybir  # noqa: E402
from concourse.bass_utils import run_bass_kernel_spmd  # noqa: E402

# Problem shapes (hardcoded per spec)
B, S, H, F, E = 4, 2048, 1024, 4096, 8
T = B * S
TOP_K = 2
N_CORES = 8
P = 128
KH = H // P          # 8  H-contraction subtiles
FT = F // P          # 32 f-tiles total
# F processed in blocks of f-tiles (weights resident per block, streamed
# double-buffered).  The first two blocks are small so the PE (strict
# FIFO queue!) never waits long for the first weight tiles: a single
# HWDGE ring delivers only ~200 GB/s, so an 8-f-tile lead block (8 MB
# fp32) would stall GEMM2-behind-GEMM1 for ~25 us at startup.
BLOCKS = (4, 4, 8, 8, 8)
NBLK = len(BLOCKS)
MH = H // P          # 8  output H tiles

F32 = mybir.dt.float32
F32R = mybir.dt.float32r
BF16 = mybir.dt.bfloat16

_CACHE: dict = {}
LAST_RESULT = None  # BassKernelResults of the most recent run (for test.py)


def _chunks_for(c_pad: int) -> tuple:
    """Token chunks: 512s with an optional single 256 tail."""
    out = [512] * (c_pad // 512)
    if c_pad % 512:
        assert c_pad % 512 == 256
        out.append(256)
    return tuple(out)


def _build(c_pad: int, chunks: tuple, use_b1: bool, mm_dt, act_fn=None):
    nc = bacc.Bacc(
        "TRN2",
        target_bir_lowering=False,
        debug=False,
        enable_asserts=False,
        num_devices=N_CORES,
    )

    xd = nc.dram_tensor("xd", [P, KH, c_pad], mm_dt, kind="ExternalInput").ap()
    w1d = nc.dram_tensor("w1d", [P, FT, KH, P], mm_dt, kind="ExternalInput").ap()
    w2d = nc.dram_tensor("w2d", [P, FT, H], mm_dt, kind="ExternalInput").ap()
    if use_b1:
        b1d = nc.dram_tensor("b1d", [P, FT], F32, kind="ExternalInput").ap()
    # per-F-block partial outputs (bf16); host sums over the NBLK axis
    yd = nc.dram_tensor(
        "yd", [P, NBLK, c_pad // P, H], BF16, kind="ExternalOutput"
    ).ap()

    gelu = act_fn or mybir.ActivationFunctionType.Gelu_apprx_tanh

    with tile.TileContext(nc) as tc:
        with (
            tc.tile_pool(name="w1p", bufs=2) as w1p,
            tc.tile_pool(name="w2p", bufs=2) as w2p,
            tc.tile_pool(name="xp", bufs=2) as xp,
            tc.tile_pool(name="hp", bufs=2) as hp,
            tc.tile_pool(name="op", bufs=8) as op,
            tc.tile_pool(name="bp", bufs=1) as bp,
            tc.tile_pool(name="ps1", bufs=3, space="PSUM") as ps1,
            tc.tile_pool(name="ps2", bufs=5, space="PSUM") as ps2,
        ):
            if use_b1:
                b1t = bp.tile([P, FT], F32)
                nc.sync.dma_start(b1t[:], b1d[:])

            # Pre-heat: ~20 throwaway matmuls on a zeroed scratch tile keep
            # the PE busy while the first weight/x DMAs land, so the HAM
            # clock-gate reaches K=8/8 (2.4 GHz) before real work starts
            # instead of paying the cold-rate double-dip observed in traces.
            scr = bp.tile([P, 512], BF16, name="scr")
            nc.vector.memset(scr[:], 0.0)
            for _ in range(12):
                wt = ps1.tile([P, 512], F32, tag="pt1")
                nc.tensor.matmul(
                    wt[:], scr[:, :P], scr[:], start=True, stop=True
                )

            fstart = 0
            for bi, fbn in enumerate(BLOCKS):
                # weights stream on the gpsimd SWDGE ring, cast bf16->fp32
                # in flight.  Few, large descriptors (SWDGE pays ~2us per
                # dma_start): two halves for W1 so the first GEMM1 matmuls
                # can start after ~1MB, one descriptor for W2.
                # Weights stream on the scalar HWDGE ring (x and y-out own
                # the sync ring; gpsimd is unusable early — Tile inserts a
                # ~20us DRAIN barrier on it at startup).  A single HWDGE
                # ring delivers ~200 GB/s and the PE queue is strict FIFO,
                # hence the small lead blocks: the ring keeps pace with the
                # PE's consumption order w1a, w1b, w2a, w2b.
                w1q = w1p.tile([P, fbn, KH, P], mm_dt, tag="w1q", name=f"w1q_{bi}")
                w2q = w2p.tile([P, fbn, H], mm_dt, tag="w2q", name=f"w2q_{bi}")
                half = fbn // 2
                nc.scalar.dma_start(w1q[:, :half], w1d[:, fstart : fstart + half])
                nc.scalar.dma_start(
                    w1q[:, half:], w1d[:, fstart + half : fstart + fbn]
                )
                nc.scalar.dma_start(
                    w2q[:, :half], w2d[:, fstart : fstart + half]
                )
                nc.scalar.dma_start(
                    w2q[:, half:], w2d[:, fstart + half : fstart + fbn]
                )

                coff = 0
                for ci, nt in enumerate(chunks):
                    xt = xp.tile([P, KH, nt], mm_dt, tag="xt")
                    if ci == 0:
                        # halves: GEMM1 k=0..3 can start on the first piece
                        nc.sync.dma_start(xt[:, :4], xd[:, :4, coff : coff + nt])
                        nc.sync.dma_start(xt[:, 4:], xd[:, 4:, coff : coff + nt])
                    else:
                        nc.sync.dma_start(xt[:], xd[:, :, coff : coff + nt])

                    # GEMM1: hT[f, :] = gelu(sum_k W1[k, f-tile].T @ xT[k, :])
                    hq = hp.tile([P, fbn, nt], mm_dt, tag="hq", name=f"hq_{bi}")
                    for f in range(fbn):
                        pt1 = ps1.tile([P, nt], F32, tag="pt1")
                        for k in range(KH):
                            nc.tensor.matmul(
                                pt1[:],
                                w1q[:, f, k, :],
                                xt[:, k, :],
                                start=(k == 0),
                                stop=(k == KH - 1),
                            )
                        bias = (
                            b1t[:, fstart + f : fstart + f + 1] if use_b1 else 0.0
                        )
                        nc.scalar.activation(hq[:, f, :], pt1[:], gelu, bias=bias)

                    # GEMM2 (partial over this F-block):
                    # Y[t-tile, hh] += sum_k2 hT[k2, t-tile].T @ W2[k2, hh]
                    for t in range(nt // P):
                        pts = [
                            ps2.tile([P, 512], F32, tag="pt2", name=f"pt2_{hh}")
                            for hh in range(2)
                        ]
                        for k2 in range(fbn):
                            for hh in range(2):
                                nc.tensor.matmul(
                                    pts[hh][:],
                                    hq[:, k2, t * P : (t + 1) * P],
                                    w2q[:, k2, hh * 512 : (hh + 1) * 512],
                                    start=(k2 == 0),
                                    stop=(k2 == fbn - 1),
                                )
                        trow = coff // P + t
                        for hh in range(2):
                            ot = op.tile([P, 512], BF16, tag="ot")
                            dst = yd[:, bi, trow, hh * 512 : (hh + 1) * 512]
                            nc.vector.tensor_copy(ot[:], pts[hh][:])
                            nc.sync.dma_start(dst, ot[:])
                    coff += nt
                fstart += fbn

    nc.compile()
    return nc


def _route(x2d, Wg):
    """Replicates reference router: softmax -> top-2 -> renormalize."""
    logits = x2d @ Wg  # [T, E] fp32
    m = logits.max(axis=-1, keepdims=True)
    p = np.exp(logits - m, dtype=np.float32)
    p /= p.sum(axis=-1, keepdims=True)
    # jax.lax.top_k: values descending, ties broken by lower index.
    order = np.argsort(-p, axis=-1, kind="stable")
    top_i = order[:, :TOP_K]  # [T, 2]
    top_p = np.take_along_axis(p, top_i, axis=-1)
    top_p = top_p / top_p.sum(axis=-1, keepdims=True)
    return top_i, top_p


def kernel(x, Wg, W1, b1, W2, b2):
    global LAST_RESULT
    x = np.ascontiguousarray(np.asarray(x, dtype=np.float32))
    Wg = np.ascontiguousarray(np.asarray(Wg, dtype=np.float32))
    W1 = np.ascontiguousarray(np.asarray(W1, dtype=np.float32))
    b1 = np.ascontiguousarray(np.asarray(b1, dtype=np.float32))
    W2 = np.ascontiguousarray(np.asarray(W2, dtype=np.float32))
    b2 = np.ascontiguousarray(np.asarray(b2, dtype=np.float32))

    x2d = x.reshape(T, H)
    top_i, top_p = _route(x2d, Wg)

    rows = [None] * E
    gval = [None] * E
    for e in range(E):
        r, slot = np.nonzero(top_i == e)
        rows[e] = r
        gval[e] = top_p[r, slot]

    c_max = max(len(r) for r in rows)
    c_pad = max(512, ((c_max + 255) // 256) * 256)
    chunks = _chunks_for(c_pad)
    use_b1 = bool(np.any(b1))

    key = (c_pad, chunks, use_b1)
    if key not in _CACHE:
        _CACHE[key] = _build(c_pad, chunks, use_b1, F32R)
    nc = _CACHE[key]

    bf16 = mybir.dt.np(BF16)
    in_maps = []
    for e in range(E):
        ce = len(rows[e])
        xt = np.zeros((H, c_pad), np.float32)
        xt[:, :ce] = x2d[rows[e]].T
        m = {
            "xd": np.ascontiguousarray(
                xt.reshape(KH, P, c_pad).transpose(1, 0, 2)
            ),
            "w1d": np.ascontiguousarray(
                W1[e].reshape(KH, P, FT, P).transpose(1, 2, 0, 3)
            ),
            "w2d": np.ascontiguousarray(
                W2[e].reshape(FT, P, H).transpose(1, 0, 2)
            ),
        }
        if use_b1:
            m["b1d"] = np.ascontiguousarray(b1[e].reshape(FT, P).T)
        in_maps.append(m)

    trace = os.environ.get("KERNEL_TRACE", "") == "1"
    res = run_bass_kernel_spmd(
        nc,
        in_maps,
        core_ids=list(range(N_CORES)),
        trace=trace,
        trace_cores=[0] if trace else None,
    )
    LAST_RESULT = res

    out = np.zeros((T, H), np.float32)
    for e in range(E):
        ce = len(rows[e])
        yt = res.results[e]["yd"].astype(np.float32).sum(axis=1)  # [P, c_pad//P, H]
        y = yt.transpose(1, 0, 2).reshape(c_pad, H)[:ce]
        out[rows[e]] += gval[e][:, None] * (y + b2[e][None, :])

    return out.reshape(B, S, H)
